# revision 16
# baseline (speedup 1.0000x reference)
"""Trainium2 Bass kernel for nn_BinaryTokenClassificationModel (segment_reduce).

Math: logits[b,i,j] = dot(segmean(1+i), w_src) + dot(segmean(513+j), w_tgt) + bias,
where segmean(s) is the mean of outputs[b] over the s-th consecutive run of equal
word_ids (attention_mask is all ones here).  dot commutes with the segment mean,
so per-token dots v[t,c] = x[t].w_c suffice; segment sums of v are accumulated by
PE one-hot matmuls and scaled by host-computed 1/count at the very end.

Design (v3, DMA-roofline oriented):
  - Only tokens of segments 1..1024 are staged (host gathers them REVERSED, so
    tgt segments 1024..513 stream first, then src 512..1).  NT = ceil(max/128)
    tiles of 128 tokens; short examples padded with slo=-1 dummies.
  - x is cast f32->bf16 during the SWDGE (gpsimd) DMA: HBM reads stay f32 (the
    mandatory roofline) but on-chip compute runs at 16-bit rates.  All x DMAs
    are issued up-front in 8 chunks so the HBM stream is continuous.
  - Per tile: DVE tensor_tensor multiplies x by the replicated weight row (bf16
    2x mode, ~690ns); the h-reduction to v is split between ACT's fused
    activation-accumulate and DVE tensor_reduce so neither engine exceeds the
    DMA stream time.  gpsimd builds the tiny per-u mask r_t = ch*v (bf16), and
    PE accumulates pool[s_lo, u] += onehot(s_lo)^T @ r_t with all-bf16 matmuls
    into small PSUM regions (tgt: u 4..8, src: u 0..4, late-src: u 0).
  - Counts never touch the device loop: host bakes 1/count into tiny [128,5]
    tables applied at the tail.  The tgt half of the output (broadcast row) and
    blocks 1-3 are emitted early, hidden under the src-phase DMA stream; only
    block 0 (which needs the last tiles) remains in the tail.
  - Output is written bf16 (tolerance 2e-2; bf16 error ~5e-3) and upcast on host.

Sharding: pure data parallel, one example (B=8) per NeuronCore (8 cores).
"""
import sys

for _p in ("/opt/trn_rl_repo", "/root/.axon_site/_ro/trn_rl_repo"):
    if _p not in sys.path:
        sys.path.append(_p)

from contextlib import ExitStack

import ml_dtypes
import numpy as np

import concourse.bacc as bacc
import concourse.bass as bass
import concourse.tile as tile
from concourse import mybir
from concourse.bass_utils import run_bass_kernel_spmd

F32 = mybir.dt.float32
BF16 = mybir.dt.bfloat16
P = 128
H = 1024
AL = mybir.AluOpType
ACTF = mybir.ActivationFunctionType

# x-tile DMA chunking (tiles per SWDGE dma_start); first chunk small so
# compute starts early, last chunks small to keep the post-stream tail short
def _chunks_for(NT):
    sizes = []
    rem = NT
    plan = [1, 2, 3, 3, 3, 3, 2]
    for s in plan:
        if rem <= 2:
            break
        k = min(s, rem - 1)
        sizes.append(k)
        rem -= k
    sizes += [1] * rem
    starts = np.cumsum([0] + sizes[:-1]).tolist()
    return list(zip(starts, sizes))


def _build_nc(NT: int, ops: list, CW: int, lt_tgt: int, lt_s1: int, lt_src: int) -> bass.Bass:
    NCF = CW + 11          # ch_all | rcnt_src | rcnt_tgt | bias
    NCB = 4 * P + NT + 8   # s1 | s2 | ident | iota | slo | zeros(8)
    nc = bacc.Bacc("TRN2", target_bir_lowering=False, debug=False, num_devices=8)
    x_d = nc.declare_dram_parameter("x", [NT * P, H], F32, isOutput=False)
    cf_d = nc.declare_dram_parameter("consts", [P, NCF], F32, isOutput=False)
    cb_d = nc.declare_dram_parameter("cbf", [P, NCB], BF16, isOutput=False)
    wb_d = nc.declare_dram_parameter("wrepb", [P, 2 * H], BF16, isOutput=False)
    y_d = nc.declare_dram_parameter("y", [512, 512], BF16, isOutput=True)

    with tile.TileContext(nc) as tc, ExitStack() as ctx:
        consts = ctx.enter_context(tc.tile_pool(name="consts", bufs=1))
        clp = ctx.enter_context(tc.tile_pool(name="clp", bufs=1))
        xpool = ctx.enter_context(tc.tile_pool(name="xp", bufs=1))
        scrp = ctx.enter_context(tc.tile_pool(name="scr", bufs=4))
        scrp2 = ctx.enter_context(tc.tile_pool(name="scr2", bufs=2))
        vpool = ctx.enter_context(tc.tile_pool(name="vp", bufs=8))
        rpool = ctx.enter_context(tc.tile_pool(name="rp", bufs=4))
        segp = ctx.enter_context(tc.tile_pool(name="segp", bufs=1))
        opool = ctx.enter_context(tc.tile_pool(name="op", bufs=4))
        pp_pool = ctx.enter_context(tc.tile_pool(name="ppool", bufs=1, space="PSUM"))
        pp_row = ctx.enter_context(tc.tile_pool(name="prow", bufs=1, space="PSUM"))
        pp_ms = ctx.enter_context(tc.tile_pool(name="pms", bufs=1, space="PSUM"))

        # ---- HWDGE (sync) queue: weights + consts, later the output ----
        wrep = consts.tile([P, 2 * H], BF16)
        nc.sync.dma_start(out=wrep[:, H:2 * H], in_=wb_d[:, H:2 * H])  # w_tgt first
        cb = consts.tile([P, NCB], BF16)
        nc.sync.dma_start(out=cb, in_=cb_d[:])
        cf = consts.tile([P, NCF], F32)
        nc.sync.dma_start(out=cf, in_=cf_d[:])
        nc.sync.dma_start(out=wrep[:, 0:H], in_=wb_d[:, 0:H])

        ch_all = cf[:, 0:CW]
        rcS = cf[:, CW:CW + 5]
        rcT = cf[:, CW + 5:CW + 10]
        biascol = cf[:, CW + 10:CW + 11]
        s1 = cb[:, 0:P]
        s2 = cb[:, P:2 * P]
        ident = cb[:, 2 * P:3 * P]
        iota = cb[:, 3 * P:4 * P]
        slo = cb[:, 4 * P:4 * P + NT]
        zeros8 = cb[:, 4 * P + NT:4 * P + NT + 8]

        # ---- PSUM pools, zero-initialized via start=True matmuls ----
        # pool_s2 takes the src contributions of tiles after lt_s1 (they only
        # touch u=0), so pool_s closes early and blocks 1-3 can be emitted
        # while the x stream is still running.
        pool_t = pp_pool.tile([P, 5], F32)  # tgt sums: col j = u-4, row = s%128
        pool_s = pp_pool.tile([P, 5], F32)  # src sums: col j = u,   row = s%128
        pool_s2 = pp_pool.tile([P, 1], F32)
        nc.tensor.matmul(pool_t, lhsT=iota, rhs=zeros8[:, 0:5], start=True,
                         stop=False, skip_group_check=True)
        nc.tensor.matmul(pool_s, lhsT=iota, rhs=zeros8[:, 0:5], start=True,
                         stop=False, skip_group_check=True)
        nc.tensor.matmul(pool_s2, lhsT=iota, rhs=zeros8[:, 0:1], start=True,
                         stop=False, skip_group_check=True)

        # ---- x stream: all chunks up-front on the SWDGE (gpsimd) queue,
        # cast f32->bf16 in the DMA datapath ----
        chunks = _chunks_for(NT)
        x_tiles = [None] * NT
        x_pair = {}  # i -> [P, 2, H] slice covering tiles i, i+1
        for c, (st, k) in enumerate(chunks):
            xc = xpool.tile([P, k, H], BF16, name=f"xc{c}")
            nc.gpsimd.dma_start(
                out=xc, in_=x_d[P * st:P * (st + k), :].rearrange("(k p) h -> p k h", p=P))
            for j in range(k):
                x_tiles[st + j] = xc[:, j, :]
                if j + 1 < k:
                    x_pair[st + j] = xc[:, j:j + 2, :]

        cl_all = clp.tile([P, NT, P], BF16)
        n_pre_cls = min(3, NT)
        nc.vector.tensor_tensor(
            out=cl_all[:, 0:n_pre_cls, :],
            in0=iota.unsqueeze(1).to_broadcast((P, n_pre_cls, P)),
            in1=slo[:, 0:n_pre_cls].unsqueeze(2).to_broadcast((P, n_pre_cls, P)),
            op=AL.is_equal)

        # ---- main loop over token tiles ----
        rowb_sb = segp.tile([P, 512], BF16)
        msrcm14 = segp.tile([P, 5], BF16)

        def emit_block(k, rhs1, rhs2):
            msps = pp_ms.tile([P, 1], F32, name=f"msps{k}")
            nc.tensor.matmul(msps, lhsT=s1, rhs=rhs1, start=True, stop=False,
                             skip_group_check=True)
            nc.tensor.matmul(msps, lhsT=s2, rhs=rhs2, start=False, stop=True,
                             skip_group_check=True)
            msv = segp.tile([P, 1], F32, name=f"msv{k}")
            nc.vector.tensor_copy(out=msv, in_=msps)
            lg = opool.tile([P, 512], BF16, name=f"lg{k}")
            nc.vector.tensor_scalar(out=lg, in0=rowb_sb, scalar1=msv,
                                    scalar2=None, op0=AL.add)
            nc.sync.dma_start(out=y_d[P * k:P * (k + 1), :], in_=lg)

        # TT pairing: adjacent same-chunk single-op tiles with equal channel
        # share one wide [P, 2, H] multiply (bf16 2x amortizes the op cost)
        paired_with = {}
        i = 0
        while i < NT - 1:
            if (i in x_pair and len(ops[i]) == 1 and len(ops[i + 1]) == 1
                    and ops[i][0]["c"] == ops[i + 1][0]["c"]):
                paired_with[i] = i + 1
                i += 2
            else:
                i += 1

        scr_of = {}
        for i in range(NT):
            if i == n_pre_cls and NT > n_pre_cls:
                # cls one-hots for the remaining tiles, slotted here so the
                # first tiles' multiplies are not stuck behind this 2.5us op
                nc.vector.tensor_tensor(
                    out=cl_all[:, n_pre_cls:NT, :],
                    in0=iota.unsqueeze(1).to_broadcast((P, NT - n_pre_cls, P)),
                    in1=slo[:, n_pre_cls:NT].unsqueeze(2).to_broadcast((P, NT - n_pre_cls, P)),
                    op=AL.is_equal)
            if i in paired_with:
                c01 = 1 if ops[i][0]["c"] == "tgt" else 0
                scr2 = scrp2.tile([P, 2, H], BF16, name="scr2")
                nc.vector.tensor_tensor(
                    out=scr2, in0=x_pair[i],
                    in1=wrep[:, c01 * H:(c01 + 1) * H].unsqueeze(1).to_broadcast((P, 2, H)),
                    op=AL.mult)
                scr_of[(i, 0)] = scr2[:, 0, :]
                scr_of[(i + 1, 0)] = scr2[:, 1, :]
            for oi, e in enumerate(ops[i]):
                c01 = 1 if e["c"] == "tgt" else 0
                if (i, oi) in scr_of:
                    scr = scr_of[(i, oi)]
                else:
                    scr = scrp.tile([P, H], BF16, name="scr1")
                    nc.vector.tensor_tensor(out=scr, in0=x_tiles[i],
                                            in1=wrep[:, c01 * H:(c01 + 1) * H], op=AL.mult)
                v = vpool.tile([P, 1], F32)
                if e["red"] == "dve":
                    nc.vector.tensor_reduce(out=v, in_=scr, axis=mybir.AxisListType.X,
                                            op=AL.add)
                else:
                    nc.scalar.activation(out=scr, in_=scr, func=ACTF.Copy, accum_out=v)
                nU = len(e["ulist"])
                r_t = rpool.tile([P, nU], BF16)
                off = e["ch_off"]
                nc.gpsimd.tensor_tensor(out=r_t, in0=ch_all[:, off:off + nU],
                                        in1=v.to_broadcast((P, nU)), op=AL.mult)
                if e["c"] == "tgt":
                    pool, col_lo, stop = pool_t, e["ulist"][0] - 4, i == lt_tgt
                elif i <= lt_s1:
                    pool, col_lo, stop = pool_s, e["ulist"][0], i == lt_s1
                else:
                    assert e["ulist"] == [0]
                    pool, col_lo, stop = pool_s2, 0, i == lt_src
                nc.tensor.matmul(pool[:, col_lo:col_lo + nU], lhsT=cl_all[:, i, :],
                                 rhs=r_t, start=False, stop=stop, skip_group_check=True)
            if i == lt_tgt:
                # tgt tail early: broadcast row of the output, hidden under the
                # src-phase DMA stream
                mtgtm = segp.tile([P, 5], BF16)
                nc.vector.tensor_tensor(out=mtgtm, in0=pool_t, in1=rcT, op=AL.mult)
                rowb_ps = pp_row.tile([P, 512], F32)
                nc.tensor.matmul(rowb_ps[:, 0:127], lhsT=mtgtm[:, 0:1].to_broadcast((P, P)),
                                 rhs=ident[:, 1:128], start=True, stop=True)
                nc.tensor.matmul(rowb_ps[:, 127:255], lhsT=mtgtm[:, 1:2].to_broadcast((P, P)),
                                 rhs=ident, start=True, stop=True)
                nc.tensor.matmul(rowb_ps[:, 255:383], lhsT=mtgtm[:, 2:3].to_broadcast((P, P)),
                                 rhs=ident, start=True, stop=True)
                nc.tensor.matmul(rowb_ps[:, 383:511], lhsT=mtgtm[:, 3:4].to_broadcast((P, P)),
                                 rhs=ident, start=True, stop=True)
                nc.tensor.matmul(rowb_ps[:, 511:512], lhsT=mtgtm[:, 4:5].to_broadcast((P, P)),
                                 rhs=ident[:, 0:1], start=True, stop=True)
                nc.scalar.activation(out=rowb_sb, in_=rowb_ps, func=ACTF.Identity,
                                     bias=biascol, scale=1.0)
            if i == lt_s1:
                # pool_s closed: blocks 1-3 emitted under the x stream
                nc.vector.tensor_tensor(out=msrcm14, in0=pool_s, in1=rcS, op=AL.mult)
                for k in (1, 2, 3):
                    emit_block(k, msrcm14[:, k:k + 1], msrcm14[:, k + 1:k + 2])

        # ---- final tail: only block 0 (needs the trailing u=0 tiles) ----
        msrcm0 = segp.tile([P, 1], BF16)
        if lt_src > lt_s1:
            nc.vector.tensor_scalar(out=msrcm0, in0=pool_s[:, 0:1], scalar1=pool_s2,
                                    scalar2=rcS[:, 0:1], op0=AL.add, op1=AL.mult)
        else:
            nc.vector.tensor_tensor(out=msrcm0, in0=pool_s[:, 0:1], in1=rcS[:, 0:1],
                                    op=AL.mult)
        emit_block(0, msrcm0, msrcm14[:, 1:2])

    nc.compile()
    return nc


def _host_prep(inputs):
    x = np.asarray(inputs["outputs"], dtype=np.float32)
    wid = np.asarray(inputs["word_ids"]).astype(np.int64)
    cw = np.asarray(inputs["classifier_w"], dtype=np.float32)
    bias = np.float32(np.asarray(inputs["classifier_b"]))
    B, L, Hd = x.shape
    assert (Hd, L, B) == (H, 4096, 8)
    assert int(inputs["num_src"]) == 512 and int(inputs["num_tgt"]) == 512
    assert np.asarray(inputs["attention_mask"]).min() == 1

    segs, idxs = [], []
    for b in range(B):
        ns = np.ones(L, np.int64)
        ns[1:] = wid[b, 1:] != wid[b, :-1]
        seg = np.cumsum(ns) - 1
        keep = (seg >= 1) & (seg <= 1024)
        idxs.append(np.nonzero(keep)[0][::-1])  # descending segment order
        segs.append(seg)
    ntoks = [len(i) for i in idxs]
    NT = (max(ntoks) + P - 1) // P
    L2 = NT * P

    tok_s = np.full((B, L2), -1, np.int64)
    xbs = []
    for b in range(B):
        n = ntoks[b]
        tok_s[b, :n] = segs[b][idxs[b]]
        xi = np.zeros(L2, np.int64)
        xi[:n] = idxs[b]
        xbs.append(np.ascontiguousarray(x[b][xi]))

    is_t = tok_s >= 513
    is_s = (tok_s >= 1) & (tok_s <= 512)
    u = np.where(tok_s >= 0, tok_s >> 7, -1)
    slo_v = np.where(tok_s >= 0, tok_s & 127, -1)

    # program metadata, unioned over cores (same compiled program everywhere)
    ops, CW, ch_cols = [], 0, []
    for i in range(NT):
        sl = slice(i * P, (i + 1) * P)
        ent = []
        for cname, m in (("tgt", is_t), ("src", is_s)):
            msk = m[:, sl]
            if not msk.any():
                continue
            uu = u[:, sl][msk]
            ulist = list(range(int(uu.min()), int(uu.max()) + 1))
            assert len(ulist) <= 3
            d = dict(c=cname, ulist=ulist, ch_off=CW, red="act")
            for uv in ulist:
                ch_cols.append((i, cname, uv))
            CW += len(ulist)
            ent.append(d)
        ops.append(ent)
    lt_tgt = max(i for i in range(NT) if any(e["c"] == "tgt" for e in ops[i]))
    lt_src = max(i for i in range(NT) if any(e["c"] == "src" for e in ops[i]))
    lt_s1 = max(i for i in range(NT)
                if any(e["c"] == "src" and max(e["ulist"]) >= 1 for e in ops[i]))
    for i in range(lt_s1 + 1, NT):
        assert all(e["c"] == "src" and e["ulist"] == [0] for e in ops[i])
    # offload some reductions from ACT to DVE so neither engine exceeds the
    # DMA stream time
    flat = [e for ent in ops for e in ent]
    n_dve = max(0, (len(flat) * 2) // 8)  # ~1/4 of reduces on DVE
    for j in np.linspace(2, len(flat) - 2, n_dve).astype(int):
        flat[int(j)]["red"] = "dve"

    iota_h = np.broadcast_to(np.arange(P, dtype=np.float32), (P, P))
    s1_h = np.eye(P, k=-1, dtype=np.float32)  # s1[p,m]=1 iff m==p-1 -> out[m]=in[m+1]
    s2_h = np.zeros((P, P), np.float32)
    s2_h[0, P - 1] = 1.0
    ident_h = np.eye(P, dtype=np.float32)
    wrep_h = np.broadcast_to(cw, (P, 2 * H)).astype(ml_dtypes.bfloat16)

    in_maps = []
    for b in range(B):
        cnt = np.bincount(tok_s[b][tok_s[b] >= 0], minlength=1025).astype(np.float64)
        rcS_h = np.ones((P, 5), np.float32)
        rcT_h = np.ones((P, 5), np.float32)
        for j in range(5):
            for p in range(P):
                s_src = 128 * j + p
                if 1 <= s_src <= 512:
                    rcS_h[p, j] = 1.0 / max(cnt[s_src], 1.0)
                s_tgt = 128 * (j + 4) + p
                if 513 <= s_tgt <= 1024:
                    rcT_h[p, j] = 1.0 / max(cnt[s_tgt], 1.0)
        slo_t = slo_v[b].reshape(NT, P).T.astype(np.float32)  # [128, NT]
        ch_h = np.zeros((P, CW), np.float32)
        for k, (i, cname, uv) in enumerate(ch_cols):
            m = (is_t if cname == "tgt" else is_s)[b, i * P:(i + 1) * P]
            ch_h[:, k] = (m & (u[b, i * P:(i + 1) * P] == uv)).astype(np.float32)
        biascol = np.full((P, 1), bias, np.float32)
        cf_h = np.concatenate([ch_h, rcS_h, rcT_h, biascol], axis=1)
        cb_h = np.concatenate(
            [s1_h, s2_h, ident_h, iota_h, slo_t, np.zeros((P, 8), np.float32)],
            axis=1).astype(ml_dtypes.bfloat16)
        in_maps.append({
            "x": xbs[b],
            "consts": np.ascontiguousarray(cf_h.astype(np.float32)),
            "cbf": np.ascontiguousarray(cb_h),
            "wrepb": np.ascontiguousarray(wrep_h),
        })
    return NT, ops, CW, lt_tgt, lt_s1, lt_src, in_maps


def _run(inputs, trace=False, tmpdir=None):
    NT, ops, CW, lt_tgt, lt_s1, lt_src, in_maps = _host_prep(inputs)
    nc = _build_nc(NT, ops, CW, lt_tgt, lt_s1, lt_src)
    res = run_bass_kernel_spmd(nc, in_maps, core_ids=list(range(8)), trace=trace, tmpdir=tmpdir)
    out = np.stack([np.asarray(r["y"]).astype(np.float32) for r in res.results])
    return out, res


def kernel(**inputs) -> np.ndarray:
    out, _ = _run(inputs, trace=False)
    return out


if __name__ == "__main__":
    # CoreSim smoke test on core 0's inputs
    import jax
    jax.config.update("jax_platforms", "cpu")
    sys.path.insert(0, "/root/problem")
    import reference as ref
    from concourse.bass_interp import CoreSim

    inputs = ref.setup_inputs()
    NT, ops, CW, lt_tgt, lt_s1, lt_src, in_maps = _host_prep(inputs)
    print("NT =", NT, "CW =", CW, "lt_tgt =", lt_tgt, "lt_s1 =", lt_s1, "lt_src =", lt_src)
    for i, ent in enumerate(ops):
        print(i, [(e["c"], e["ulist"], e["red"]) for e in ent])
    nc = _build_nc(NT, ops, CW, lt_tgt, lt_s1, lt_src)
    sim = CoreSim(nc)
    for name, arr in in_maps[0].items():
        sim.tensor(name)[:] = arr
    sim.simulate()
    got = np.array(sim.tensor("y")).astype(np.float32)
    expected = np.asarray(ref.reference(**inputs))[0]
    err = np.abs(got - expected).max()
    scale = np.abs(expected).max()
    print("CoreSim abs err:", err, "rel:", err / scale)
    assert err / scale < 1e-2, "CoreSim mismatch"
    print("CORESIM PASSES")


# revision 19
# speedup vs baseline: 1.0681x; 1.0681x over previous
"""Trainium2 Bass kernel for nn_BinaryTokenClassificationModel (segment_reduce).

Math: logits[b,i,j] = dot(segmean(1+i), w_src) + dot(segmean(513+j), w_tgt) + bias,
where segmean(s) is the mean of outputs[b] over the s-th consecutive run of equal
word_ids (attention_mask is all ones here).  dot commutes with the segment mean,
so per-token dots v[t,c] = x[t].w_c suffice; segment sums of v are accumulated by
PE one-hot matmuls and scaled by host-computed 1/count at the very end.

Design (v3, DMA-roofline oriented):
  - Only tokens of segments 1..1024 are staged (host gathers them REVERSED, so
    tgt segments 1024..513 stream first, then src 512..1).  NT = ceil(max/128)
    tiles of 128 tokens; short examples padded with slo=-1 dummies.
  - x is cast f32->bf16 during the SWDGE (gpsimd) DMA: HBM reads stay f32 (the
    mandatory roofline) but on-chip compute runs at 16-bit rates.  All x DMAs
    are issued up-front in 8 chunks so the HBM stream is continuous.
  - Per tile: DVE tensor_tensor multiplies x by the replicated weight row (bf16
    2x mode, ~690ns); the h-reduction to v is split between ACT's fused
    activation-accumulate and DVE tensor_reduce so neither engine exceeds the
    DMA stream time.  gpsimd builds the tiny per-u mask r_t = ch*v (bf16), and
    PE accumulates pool[s_lo, u] += onehot(s_lo)^T @ r_t with all-bf16 matmuls
    into small PSUM regions (tgt: u 4..8, src: u 0..4, late-src: u 0).
  - Counts never touch the device loop: host bakes 1/count into tiny [128,5]
    tables applied at the tail.  The tgt half of the output (broadcast row) and
    blocks 1-3 are emitted early, hidden under the src-phase DMA stream; only
    block 0 (which needs the last tiles) remains in the tail.
  - Output is written bf16 (tolerance 2e-2; bf16 error ~5e-3) and upcast on host.

Sharding: pure data parallel, one example (B=8) per NeuronCore (8 cores).
"""
import sys

for _p in ("/opt/trn_rl_repo", "/root/.axon_site/_ro/trn_rl_repo"):
    if _p not in sys.path:
        sys.path.append(_p)

from contextlib import ExitStack

import ml_dtypes
import numpy as np

import concourse.bacc as bacc
import concourse.bass as bass
import concourse.tile as tile
from concourse import mybir
from concourse.bass_utils import run_bass_kernel_spmd

F32 = mybir.dt.float32
BF16 = mybir.dt.bfloat16
P = 128
H = 1024
AL = mybir.AluOpType
ACTF = mybir.ActivationFunctionType

# x-tile DMA chunking (tiles per SWDGE dma_start); first chunk small so
# compute starts early, last chunks small to keep the post-stream tail short
def _chunks_for(NT):
    sizes = []
    rem = NT
    plan = [1, 2, 3, 3, 3, 3, 2]
    for s in plan:
        if rem <= 2:
            break
        k = min(s, rem - 1)
        sizes.append(k)
        rem -= k
    sizes += [1] * rem
    starts = np.cumsum([0] + sizes[:-1]).tolist()
    return list(zip(starts, sizes))


def _build_nc(NT: int, ops: list, CW: int, lt_tgt: int, lt_s1: int, lt_src: int) -> bass.Bass:
    NCF = CW + 11          # ch_all | rcnt_src | rcnt_tgt | bias
    NCB = 4 * P + NT + 8   # s1 | s2 | ident | iota | slo | zeros(8)
    nc = bacc.Bacc("TRN2", target_bir_lowering=False, debug=False, num_devices=8)
    x_d = nc.declare_dram_parameter("x", [NT * P, H], F32, isOutput=False)
    cf_d = nc.declare_dram_parameter("consts", [P, NCF], F32, isOutput=False)
    cb_d = nc.declare_dram_parameter("cbf", [P, NCB], BF16, isOutput=False)
    wb_d = nc.declare_dram_parameter("wrepb", [P, 2 * H], BF16, isOutput=False)
    y_d = nc.declare_dram_parameter("y", [512, 512], BF16, isOutput=True)

    with tile.TileContext(nc) as tc, ExitStack() as ctx:
        consts = ctx.enter_context(tc.tile_pool(name="consts", bufs=1))
        clp = ctx.enter_context(tc.tile_pool(name="clp", bufs=1))
        xpool = ctx.enter_context(tc.tile_pool(name="xp", bufs=1))
        scrp = ctx.enter_context(tc.tile_pool(name="scr", bufs=6))
        scrp2 = ctx.enter_context(tc.tile_pool(name="scr2", bufs=4))
        vpool = ctx.enter_context(tc.tile_pool(name="vp", bufs=12))
        rpool = ctx.enter_context(tc.tile_pool(name="rp", bufs=8))
        segp = ctx.enter_context(tc.tile_pool(name="segp", bufs=1))
        opool = ctx.enter_context(tc.tile_pool(name="op", bufs=4))
        pp_pool = ctx.enter_context(tc.tile_pool(name="ppool", bufs=1, space="PSUM"))
        pp_row = ctx.enter_context(tc.tile_pool(name="prow", bufs=1, space="PSUM"))
        pp_ms = ctx.enter_context(tc.tile_pool(name="pms", bufs=1, space="PSUM"))

        # ---- HWDGE (sync) queue: weights + consts, later the output ----
        wrep = consts.tile([P, 2 * H], BF16)
        nc.sync.dma_start(out=wrep[:, H:2 * H], in_=wb_d[:, H:2 * H])  # w_tgt first
        cb = consts.tile([P, NCB], BF16)
        nc.sync.dma_start(out=cb, in_=cb_d[:])
        cf = consts.tile([P, NCF], F32)
        nc.sync.dma_start(out=cf, in_=cf_d[:])
        nc.sync.dma_start(out=wrep[:, 0:H], in_=wb_d[:, 0:H])

        ch_all = cf[:, 0:CW]
        rcS = cf[:, CW:CW + 5]
        rcT = cf[:, CW + 5:CW + 10]
        biascol = cf[:, CW + 10:CW + 11]
        s1 = cb[:, 0:P]
        s2 = cb[:, P:2 * P]
        ident = cb[:, 2 * P:3 * P]
        iota = cb[:, 3 * P:4 * P]
        slo = cb[:, 4 * P:4 * P + NT]
        zeros8 = cb[:, 4 * P + NT:4 * P + NT + 8]

        # ---- PSUM pools, zero-initialized via start=True matmuls ----
        # pool_s2 takes the src contributions of tiles after lt_s1 (they only
        # touch u=0), so pool_s closes early and blocks 1-3 can be emitted
        # while the x stream is still running.
        pool_t = pp_pool.tile([P, 5], F32)  # tgt sums: col j = u-4, row = s%128
        pool_s = pp_pool.tile([P, 5], F32)  # src sums: col j = u,   row = s%128
        pool_s2 = pp_pool.tile([P, 1], F32)
        nc.tensor.matmul(pool_t, lhsT=iota, rhs=zeros8[:, 0:5], start=True,
                         stop=False, skip_group_check=True)
        nc.tensor.matmul(pool_s, lhsT=iota, rhs=zeros8[:, 0:5], start=True,
                         stop=False, skip_group_check=True)
        nc.tensor.matmul(pool_s2, lhsT=iota, rhs=zeros8[:, 0:1], start=True,
                         stop=False, skip_group_check=True)

        # ---- x stream: all chunks up-front on the SWDGE (gpsimd) queue,
        # cast f32->bf16 in the DMA datapath ----
        chunks = _chunks_for(NT)
        x_tiles = [None] * NT
        x_pair = {}  # i -> [P, 2, H] slice covering tiles i, i+1
        for c, (st, k) in enumerate(chunks):
            xc = xpool.tile([P, k, H], BF16, name=f"xc{c}")
            nc.gpsimd.dma_start(
                out=xc, in_=x_d[P * st:P * (st + k), :].rearrange("(k p) h -> p k h", p=P))
            for j in range(k):
                x_tiles[st + j] = xc[:, j, :]
                if j + 1 < k:
                    x_pair[st + j] = xc[:, j:j + 2, :]

        cl_all = clp.tile([P, NT, P], BF16)
        n_pre_cls = min(3, NT)

        def emit_pre_cls():
            nc.vector.tensor_tensor(
                out=cl_all[:, 0:n_pre_cls, :],
                in0=iota.unsqueeze(1).to_broadcast((P, n_pre_cls, P)),
                in1=slo[:, 0:n_pre_cls].unsqueeze(2).to_broadcast((P, n_pre_cls, P)),
                op=AL.is_equal)

        # ---- main loop over token tiles ----
        rowb_sb = segp.tile([P, 512], BF16)
        msrcm14 = segp.tile([P, 5], BF16)

        def emit_block(k, rhs1, rhs2):
            msps = pp_ms.tile([P, 1], F32, name=f"msps{k}")
            nc.tensor.matmul(msps, lhsT=s1, rhs=rhs1, start=True, stop=False,
                             skip_group_check=True)
            nc.tensor.matmul(msps, lhsT=s2, rhs=rhs2, start=False, stop=True,
                             skip_group_check=True)
            msv = segp.tile([P, 1], F32, name=f"msv{k}")
            nc.vector.tensor_copy(out=msv, in_=msps)
            lg = opool.tile([P, 512], BF16, name=f"lg{k}")
            nc.vector.tensor_scalar(out=lg, in0=rowb_sb, scalar1=msv,
                                    scalar2=None, op0=AL.add)
            nc.sync.dma_start(out=y_d[P * k:P * (k + 1), :], in_=lg)

        # TT pairing: adjacent same-chunk single-op tiles with equal channel
        # share one wide [P, 2, H] multiply (bf16 2x amortizes the op cost)
        paired_with = {}
        i = 0
        while i < NT - 1:
            if (i in x_pair and len(ops[i]) == 1 and len(ops[i + 1]) == 1
                    and ops[i][0]["c"] == ops[i + 1][0]["c"]):
                paired_with[i] = i + 1
                i += 2
            else:
                i += 1

        scr_of = {}
        for i in range(NT):
            if i == n_pre_cls and NT > n_pre_cls:
                # cls one-hots for the remaining tiles, slotted here so the
                # first tiles' multiplies are not stuck behind this 2.5us op
                nc.vector.tensor_tensor(
                    out=cl_all[:, n_pre_cls:NT, :],
                    in0=iota.unsqueeze(1).to_broadcast((P, NT - n_pre_cls, P)),
                    in1=slo[:, n_pre_cls:NT].unsqueeze(2).to_broadcast((P, NT - n_pre_cls, P)),
                    op=AL.is_equal)
            if i in paired_with:
                c01 = 1 if ops[i][0]["c"] == "tgt" else 0
                scr2 = scrp2.tile([P, 2, H], BF16, name="scr2")
                nc.vector.tensor_tensor(
                    out=scr2, in0=x_pair[i],
                    in1=wrep[:, c01 * H:(c01 + 1) * H].unsqueeze(1).to_broadcast((P, 2, H)),
                    op=AL.mult)
                scr_of[(i, 0)] = scr2[:, 0, :]
                scr_of[(i + 1, 0)] = scr2[:, 1, :]
            for oi, e in enumerate(ops[i]):
                c01 = 1 if e["c"] == "tgt" else 0
                if (i, oi) in scr_of:
                    scr = scr_of[(i, oi)]
                else:
                    scr = scrp.tile([P, H], BF16, name="scr1")
                    nc.vector.tensor_tensor(out=scr, in0=x_tiles[i],
                                            in1=wrep[:, c01 * H:(c01 + 1) * H], op=AL.mult)
                if i == 0 and oi == 0:
                    # cls for the first tiles, behind tile-0's multiply so that
                    # multiply is not stuck waiting on the consts DMA
                    emit_pre_cls()
                v = vpool.tile([P, 1], F32)
                if e["red"] == "dve":
                    nc.vector.tensor_reduce(out=v, in_=scr, axis=mybir.AxisListType.X,
                                            op=AL.add)
                else:
                    nc.scalar.activation(out=scr, in_=scr, func=ACTF.Copy, accum_out=v)
                nU = len(e["ulist"])
                r_t = rpool.tile([P, nU], BF16)
                off = e["ch_off"]
                nc.gpsimd.tensor_tensor(out=r_t, in0=ch_all[:, off:off + nU],
                                        in1=v.to_broadcast((P, nU)), op=AL.mult)
                if e["c"] == "tgt":
                    pool, col_lo, stop = pool_t, e["ulist"][0] - 4, i == lt_tgt
                elif i <= lt_s1:
                    pool, col_lo, stop = pool_s, e["ulist"][0], i == lt_s1
                else:
                    assert e["ulist"] == [0]
                    pool, col_lo, stop = pool_s2, 0, i == lt_src
                nc.tensor.matmul(pool[:, col_lo:col_lo + nU], lhsT=cl_all[:, i, :],
                                 rhs=r_t, start=False, stop=stop, skip_group_check=True)
            if i == lt_tgt:
                # tgt tail early: broadcast row of the output, hidden under the
                # src-phase DMA stream
                mtgtm = segp.tile([P, 5], BF16)
                nc.vector.tensor_tensor(out=mtgtm, in0=pool_t, in1=rcT, op=AL.mult)
                rowb_ps = pp_row.tile([P, 512], F32)
                nc.tensor.matmul(rowb_ps[:, 0:127], lhsT=mtgtm[:, 0:1].to_broadcast((P, P)),
                                 rhs=ident[:, 1:128], start=True, stop=True)
                nc.tensor.matmul(rowb_ps[:, 127:255], lhsT=mtgtm[:, 1:2].to_broadcast((P, P)),
                                 rhs=ident, start=True, stop=True)
                nc.tensor.matmul(rowb_ps[:, 255:383], lhsT=mtgtm[:, 2:3].to_broadcast((P, P)),
                                 rhs=ident, start=True, stop=True)
                nc.tensor.matmul(rowb_ps[:, 383:511], lhsT=mtgtm[:, 3:4].to_broadcast((P, P)),
                                 rhs=ident, start=True, stop=True)
                nc.tensor.matmul(rowb_ps[:, 511:512], lhsT=mtgtm[:, 4:5].to_broadcast((P, P)),
                                 rhs=ident[:, 0:1], start=True, stop=True)
                nc.scalar.activation(out=rowb_sb, in_=rowb_ps, func=ACTF.Identity,
                                     bias=biascol, scale=1.0)
            if i == lt_s1:
                # pool_s closed: blocks 1-3 emitted under the x stream
                nc.vector.tensor_tensor(out=msrcm14, in0=pool_s, in1=rcS, op=AL.mult)
                for k in (1, 2, 3):
                    emit_block(k, msrcm14[:, k:k + 1], msrcm14[:, k + 1:k + 2])

        # ---- final tail: only block 0 (needs the trailing u=0 tiles) ----
        msrcm0 = segp.tile([P, 1], BF16)
        if lt_src > lt_s1:
            nc.vector.tensor_scalar(out=msrcm0, in0=pool_s[:, 0:1], scalar1=pool_s2,
                                    scalar2=rcS[:, 0:1], op0=AL.add, op1=AL.mult)
        else:
            nc.vector.tensor_tensor(out=msrcm0, in0=pool_s[:, 0:1], in1=rcS[:, 0:1],
                                    op=AL.mult)
        emit_block(0, msrcm0, msrcm14[:, 1:2])

    nc.compile()
    return nc


def _host_prep(inputs):
    x = np.asarray(inputs["outputs"], dtype=np.float32)
    wid = np.asarray(inputs["word_ids"]).astype(np.int64)
    cw = np.asarray(inputs["classifier_w"], dtype=np.float32)
    bias = np.float32(np.asarray(inputs["classifier_b"]))
    B, L, Hd = x.shape
    assert (Hd, L, B) == (H, 4096, 8)
    assert int(inputs["num_src"]) == 512 and int(inputs["num_tgt"]) == 512
    assert np.asarray(inputs["attention_mask"]).min() == 1

    segs, idxs = [], []
    for b in range(B):
        ns = np.ones(L, np.int64)
        ns[1:] = wid[b, 1:] != wid[b, :-1]
        seg = np.cumsum(ns) - 1
        keep = (seg >= 1) & (seg <= 1024)
        idxs.append(np.nonzero(keep)[0][::-1])  # descending segment order
        segs.append(seg)
    ntoks = [len(i) for i in idxs]
    NT = (max(ntoks) + P - 1) // P
    L2 = NT * P

    tok_s = np.full((B, L2), -1, np.int64)
    xbs = []
    for b in range(B):
        n = ntoks[b]
        tok_s[b, :n] = segs[b][idxs[b]]
        xi = np.zeros(L2, np.int64)
        xi[:n] = idxs[b]
        xbs.append(np.ascontiguousarray(x[b][xi]))

    is_t = tok_s >= 513
    is_s = (tok_s >= 1) & (tok_s <= 512)
    u = np.where(tok_s >= 0, tok_s >> 7, -1)
    slo_v = np.where(tok_s >= 0, tok_s & 127, -1)

    # program metadata, unioned over cores (same compiled program everywhere)
    ops, CW, ch_cols = [], 0, []
    for i in range(NT):
        sl = slice(i * P, (i + 1) * P)
        ent = []
        for cname, m in (("tgt", is_t), ("src", is_s)):
            msk = m[:, sl]
            if not msk.any():
                continue
            uu = u[:, sl][msk]
            ulist = list(range(int(uu.min()), int(uu.max()) + 1))
            assert len(ulist) <= 3
            d = dict(c=cname, ulist=ulist, ch_off=CW, red="act")
            for uv in ulist:
                ch_cols.append((i, cname, uv))
            CW += len(ulist)
            ent.append(d)
        ops.append(ent)
    lt_tgt = max(i for i in range(NT) if any(e["c"] == "tgt" for e in ops[i]))
    lt_src = max(i for i in range(NT) if any(e["c"] == "src" for e in ops[i]))
    lt_s1 = max(i for i in range(NT)
                if any(e["c"] == "src" and max(e["ulist"]) >= 1 for e in ops[i]))
    for i in range(lt_s1 + 1, NT):
        assert all(e["c"] == "src" and e["ulist"] == [0] for e in ops[i])
    # offload some reductions from ACT to DVE so neither engine exceeds the
    # DMA stream time
    flat = [e for ent in ops for e in ent]
    n = len(flat)
    # DVE takes ~6 reductions, placed late so ACT's backlog drains before the
    # x stream ends (the final op stays on ACT: it is idle by then)
    for j in [n - 10, n - 8, n - 6, n - 4, n - 3, n - 2]:
        if 0 <= j < n - 1:
            flat[j]["red"] = "dve"

    iota_h = np.broadcast_to(np.arange(P, dtype=np.float32), (P, P))
    s1_h = np.eye(P, k=-1, dtype=np.float32)  # s1[p,m]=1 iff m==p-1 -> out[m]=in[m+1]
    s2_h = np.zeros((P, P), np.float32)
    s2_h[0, P - 1] = 1.0
    ident_h = np.eye(P, dtype=np.float32)
    wrep_h = np.broadcast_to(cw, (P, 2 * H)).astype(ml_dtypes.bfloat16)

    in_maps = []
    for b in range(B):
        cnt = np.bincount(tok_s[b][tok_s[b] >= 0], minlength=1025).astype(np.float64)
        rcS_h = np.ones((P, 5), np.float32)
        rcT_h = np.ones((P, 5), np.float32)
        for j in range(5):
            for p in range(P):
                s_src = 128 * j + p
                if 1 <= s_src <= 512:
                    rcS_h[p, j] = 1.0 / max(cnt[s_src], 1.0)
                s_tgt = 128 * (j + 4) + p
                if 513 <= s_tgt <= 1024:
                    rcT_h[p, j] = 1.0 / max(cnt[s_tgt], 1.0)
        slo_t = slo_v[b].reshape(NT, P).T.astype(np.float32)  # [128, NT]
        ch_h = np.zeros((P, CW), np.float32)
        for k, (i, cname, uv) in enumerate(ch_cols):
            m = (is_t if cname == "tgt" else is_s)[b, i * P:(i + 1) * P]
            ch_h[:, k] = (m & (u[b, i * P:(i + 1) * P] == uv)).astype(np.float32)
        biascol = np.full((P, 1), bias, np.float32)
        cf_h = np.concatenate([ch_h, rcS_h, rcT_h, biascol], axis=1)
        cb_h = np.concatenate(
            [s1_h, s2_h, ident_h, iota_h, slo_t, np.zeros((P, 8), np.float32)],
            axis=1).astype(ml_dtypes.bfloat16)
        in_maps.append({
            "x": xbs[b],
            "consts": np.ascontiguousarray(cf_h.astype(np.float32)),
            "cbf": np.ascontiguousarray(cb_h),
            "wrepb": np.ascontiguousarray(wrep_h),
        })
    return NT, ops, CW, lt_tgt, lt_s1, lt_src, in_maps


def _run(inputs, trace=False, tmpdir=None):
    NT, ops, CW, lt_tgt, lt_s1, lt_src, in_maps = _host_prep(inputs)
    nc = _build_nc(NT, ops, CW, lt_tgt, lt_s1, lt_src)
    res = run_bass_kernel_spmd(nc, in_maps, core_ids=list(range(8)), trace=trace, tmpdir=tmpdir)
    out = np.stack([np.asarray(r["y"]).astype(np.float32) for r in res.results])
    return out, res


def kernel(**inputs) -> np.ndarray:
    out, _ = _run(inputs, trace=False)
    return out


if __name__ == "__main__":
    # CoreSim smoke test on core 0's inputs
    import jax
    jax.config.update("jax_platforms", "cpu")
    sys.path.insert(0, "/root/problem")
    import reference as ref
    from concourse.bass_interp import CoreSim

    inputs = ref.setup_inputs()
    NT, ops, CW, lt_tgt, lt_s1, lt_src, in_maps = _host_prep(inputs)
    print("NT =", NT, "CW =", CW, "lt_tgt =", lt_tgt, "lt_s1 =", lt_s1, "lt_src =", lt_src)
    for i, ent in enumerate(ops):
        print(i, [(e["c"], e["ulist"], e["red"]) for e in ent])
    nc = _build_nc(NT, ops, CW, lt_tgt, lt_s1, lt_src)
    sim = CoreSim(nc)
    for name, arr in in_maps[0].items():
        sim.tensor(name)[:] = arr
    sim.simulate()
    got = np.array(sim.tensor("y")).astype(np.float32)
    expected = np.asarray(ref.reference(**inputs))[0]
    err = np.abs(got - expected).max()
    scale = np.abs(expected).max()
    print("CoreSim abs err:", err, "rel:", err / scale)
    assert err / scale < 1e-2, "CoreSim mismatch"
    print("CORESIM PASSES")


# revision 22
# speedup vs baseline: 1.0930x; 1.0233x over previous
"""Trainium2 Bass kernel for nn_BinaryTokenClassificationModel (segment_reduce).

Math: logits[b,i,j] = dot(segmean(1+i), w_src) + dot(segmean(513+j), w_tgt) + bias,
where segmean(s) is the mean of outputs[b] over the s-th consecutive run of equal
word_ids (attention_mask is all ones here).  dot commutes with the segment mean,
so per-token dots v[t,c] = x[t].w_c suffice; segment sums of v are accumulated by
PE one-hot matmuls and scaled by host-computed 1/count at the very end.

Design (v3, DMA-roofline oriented):
  - Only tokens of segments 1..1024 are staged (host gathers them REVERSED, so
    tgt segments 1024..513 stream first, then src 512..1).  NT = ceil(max/128)
    tiles of 128 tokens; short examples padded with slo=-1 dummies.
  - x is cast f32->bf16 during the SWDGE (gpsimd) DMA: HBM reads stay f32 (the
    mandatory roofline) but on-chip compute runs at 16-bit rates.  All x DMAs
    are issued up-front in 8 chunks so the HBM stream is continuous.
  - Per tile: DVE tensor_tensor multiplies x by the replicated weight row (bf16
    2x mode, ~690ns); the h-reduction to v is split between ACT's fused
    activation-accumulate and DVE tensor_reduce so neither engine exceeds the
    DMA stream time.  gpsimd builds the tiny per-u mask r_t = ch*v (bf16), and
    PE accumulates pool[s_lo, u] += onehot(s_lo)^T @ r_t with all-bf16 matmuls
    into small PSUM regions (tgt: u 4..8, src: u 0..4, late-src: u 0).
  - Counts never touch the device loop: host bakes 1/count into tiny [128,5]
    tables applied at the tail.  The tgt half of the output (broadcast row) and
    blocks 1-3 are emitted early, hidden under the src-phase DMA stream; only
    block 0 (which needs the last tiles) remains in the tail.
  - Output is written bf16 (tolerance 2e-2; bf16 error ~5e-3) and upcast on host.

Sharding: pure data parallel, one example (B=8) per NeuronCore (8 cores).
"""
import sys

for _p in ("/opt/trn_rl_repo", "/root/.axon_site/_ro/trn_rl_repo"):
    if _p not in sys.path:
        sys.path.append(_p)

from contextlib import ExitStack

import ml_dtypes
import numpy as np

import concourse.bacc as bacc
import concourse.bass as bass
import concourse.tile as tile
from concourse import mybir
from concourse.bass_utils import run_bass_kernel_spmd

F32 = mybir.dt.float32
BF16 = mybir.dt.bfloat16
P = 128
H = 1024
AL = mybir.AluOpType
ACTF = mybir.ActivationFunctionType

# x-tile DMA chunking (tiles per SWDGE dma_start); first chunk small so
# compute starts early, last chunks small to keep the post-stream tail short
def _chunks_for(NT):
    sizes = []
    rem = NT
    plan = [1, 2, 3, 3, 3, 3, 2]
    for s in plan:
        if rem <= 2:
            break
        k = min(s, rem - 1)
        sizes.append(k)
        rem -= k
    sizes += [1] * rem
    starts = np.cumsum([0] + sizes[:-1]).tolist()
    return list(zip(starts, sizes))


def _build_nc(NT: int, ops: list, CW: int, lt_tgt: int, lt_s1: int, lt_src: int) -> bass.Bass:
    NCF = CW + 11          # ch_all | rcnt_src | rcnt_tgt | bias
    NCB = 4 * P + NT + 8   # s1 | s2 | ident | iota | slo | zeros(8)
    nc = bacc.Bacc("TRN2", target_bir_lowering=False, debug=False, num_devices=8)
    x_d = nc.declare_dram_parameter("x", [NT * P, H], F32, isOutput=False)
    cf_d = nc.declare_dram_parameter("consts", [P, NCF], F32, isOutput=False)
    cb_d = nc.declare_dram_parameter("cbf", [P, NCB], BF16, isOutput=False)
    wb_d = nc.declare_dram_parameter("wrepb", [P, 2 * H], BF16, isOutput=False)
    y_d = nc.declare_dram_parameter("y", [512, 512], BF16, isOutput=True)

    with tile.TileContext(nc) as tc, ExitStack() as ctx:
        consts = ctx.enter_context(tc.tile_pool(name="consts", bufs=1))
        clp = ctx.enter_context(tc.tile_pool(name="clp", bufs=1))
        xpool = ctx.enter_context(tc.tile_pool(name="xp", bufs=1))
        scrp = ctx.enter_context(tc.tile_pool(name="scr", bufs=6))
        scrp2 = ctx.enter_context(tc.tile_pool(name="scr2", bufs=4))
        vpool = ctx.enter_context(tc.tile_pool(name="vp", bufs=12))
        rpool = ctx.enter_context(tc.tile_pool(name="rp", bufs=8))
        segp = ctx.enter_context(tc.tile_pool(name="segp", bufs=1))
        opool = ctx.enter_context(tc.tile_pool(name="op", bufs=4))
        pp_pool = ctx.enter_context(tc.tile_pool(name="ppool", bufs=1, space="PSUM"))
        pp_row = ctx.enter_context(tc.tile_pool(name="prow", bufs=1, space="PSUM"))
        pp_ms = ctx.enter_context(tc.tile_pool(name="pms", bufs=1, space="PSUM"))

        # ---- HWDGE (sync) queue: weights + consts, later the output ----
        wrep = consts.tile([P, 2 * H], BF16)
        nc.sync.dma_start(out=wrep[:, H:2 * H], in_=wb_d[:, H:2 * H])  # w_tgt first
        cb = consts.tile([P, NCB], BF16)
        nc.sync.dma_start(out=cb, in_=cb_d[:])
        cf = consts.tile([P, NCF], F32)
        nc.sync.dma_start(out=cf, in_=cf_d[:])
        nc.sync.dma_start(out=wrep[:, 0:H], in_=wb_d[:, 0:H])

        ch_all = cf[:, 0:CW]
        rcS = cf[:, CW:CW + 5]
        rcT = cf[:, CW + 5:CW + 10]
        biascol = cf[:, CW + 10:CW + 11]
        s1 = cb[:, 0:P]
        s2 = cb[:, P:2 * P]
        ident = cb[:, 2 * P:3 * P]
        iota = cb[:, 3 * P:4 * P]
        slo = cb[:, 4 * P:4 * P + NT]
        zeros8 = cb[:, 4 * P + NT:4 * P + NT + 8]

        # ---- PSUM pools, zero-initialized via start=True matmuls ----
        # pool_s2 takes the src contributions of tiles after lt_s1 (they only
        # touch u=0), so pool_s closes early and blocks 1-3 can be emitted
        # while the x stream is still running.
        pool_t = pp_pool.tile([P, 5], F32)  # tgt sums: col j = u-4, row = s%128
        pool_s = pp_pool.tile([P, 5], F32)  # src sums: col j = u,   row = s%128
        pool_s2 = pp_pool.tile([P, 1], F32)
        nc.tensor.matmul(pool_t, lhsT=iota, rhs=zeros8[:, 0:5], start=True,
                         stop=False, skip_group_check=True)
        nc.tensor.matmul(pool_s, lhsT=iota, rhs=zeros8[:, 0:5], start=True,
                         stop=False, skip_group_check=True)
        nc.tensor.matmul(pool_s2, lhsT=iota, rhs=zeros8[:, 0:1], start=True,
                         stop=False, skip_group_check=True)

        # ---- x stream: all chunks up-front on the SWDGE (gpsimd) queue,
        # cast f32->bf16 in the DMA datapath ----
        chunks = _chunks_for(NT)
        x_tiles = [None] * NT
        x_chunk = {}  # chunk start tile -> whole-chunk SBUF tile
        for c, (st, k) in enumerate(chunks):
            xc = xpool.tile([P, k, H], BF16, name=f"xc{c}")
            nc.gpsimd.dma_start(
                out=xc, in_=x_d[P * st:P * (st + k), :].rearrange("(k p) h -> p k h", p=P))
            x_chunk[st] = xc
            for j in range(k):
                x_tiles[st + j] = xc[:, j, :]

        cl_all = clp.tile([P, NT, P], BF16)
        n_pre_cls = min(3, NT)

        def emit_pre_cls():
            nc.vector.tensor_tensor(
                out=cl_all[:, 0:n_pre_cls, :],
                in0=iota.unsqueeze(1).to_broadcast((P, n_pre_cls, P)),
                in1=slo[:, 0:n_pre_cls].unsqueeze(2).to_broadcast((P, n_pre_cls, P)),
                op=AL.is_equal)

        # ---- main loop over token tiles ----
        rowb_sb = segp.tile([P, 512], BF16)
        msrcm14 = segp.tile([P, 5], BF16)

        def emit_block(k, rhs1, rhs2):
            msps = pp_ms.tile([P, 1], F32, name=f"msps{k}")
            nc.tensor.matmul(msps, lhsT=s1, rhs=rhs1, start=True, stop=False,
                             skip_group_check=True)
            nc.tensor.matmul(msps, lhsT=s2, rhs=rhs2, start=False, stop=True,
                             skip_group_check=True)
            msv = segp.tile([P, 1], F32, name=f"msv{k}")
            nc.vector.tensor_copy(out=msv, in_=msps)
            lg = opool.tile([P, 512], BF16, name=f"lg{k}")
            nc.vector.tensor_scalar(out=lg, in0=rowb_sb, scalar1=msv,
                                    scalar2=None, op0=AL.add)
            nc.sync.dma_start(out=y_d[P * k:P * (k + 1), :], in_=lg)

        # TT grouping: maximal runs of single-op same-channel tiles within one
        # chunk share one wide [P, k, H] multiply (amortizes per-op cost)
        group_at = {}   # first tile -> run length
        for st, k in chunks:
            j = 0
            while j < k:
                i0 = st + j
                r = 1
                if len(ops[i0]) == 1:
                    while (j + r < k and len(ops[st + j + r]) == 1
                           and ops[st + j + r][0]["c"] == ops[i0][0]["c"]):
                        r += 1
                if r > 1:
                    group_at[i0] = (r, st, j)
                j += r

        scr_of = {}
        for i in range(NT):
            if i == n_pre_cls and NT > n_pre_cls:
                # cls one-hots for the remaining tiles, slotted here so the
                # first tiles' multiplies are not stuck behind this 2.5us op
                nc.vector.tensor_tensor(
                    out=cl_all[:, n_pre_cls:NT, :],
                    in0=iota.unsqueeze(1).to_broadcast((P, NT - n_pre_cls, P)),
                    in1=slo[:, n_pre_cls:NT].unsqueeze(2).to_broadcast((P, NT - n_pre_cls, P)),
                    op=AL.is_equal)
            if i in group_at:
                r, st, j = group_at[i]
                c01 = 1 if ops[i][0]["c"] == "tgt" else 0
                scr2 = scrp2.tile([P, 3, H], BF16, name="scr2")
                nc.vector.tensor_tensor(
                    out=scr2[:, 0:r, :], in0=x_chunk[st][:, j:j + r, :],
                    in1=wrep[:, c01 * H:(c01 + 1) * H].unsqueeze(1).to_broadcast((P, r, H)),
                    op=AL.mult)
                for q in range(r):
                    scr_of[(i + q, 0)] = scr2[:, q, :]
            for oi, e in enumerate(ops[i]):
                c01 = 1 if e["c"] == "tgt" else 0
                if (i, oi) in scr_of:
                    scr = scr_of[(i, oi)]
                else:
                    scr = scrp.tile([P, H], BF16, name="scr1")
                    nc.vector.tensor_tensor(out=scr, in0=x_tiles[i],
                                            in1=wrep[:, c01 * H:(c01 + 1) * H], op=AL.mult)
                if i == 0 and oi == 0:
                    # cls for the first tiles, behind tile-0's multiply so that
                    # multiply is not stuck waiting on the consts DMA
                    emit_pre_cls()
                v = vpool.tile([P, 1], F32)
                if e["red"] == "dve":
                    nc.vector.tensor_reduce(out=v, in_=scr, axis=mybir.AxisListType.X,
                                            op=AL.add)
                else:
                    nc.scalar.activation(out=scr, in_=scr, func=ACTF.Copy, accum_out=v)
                nU = len(e["ulist"])
                r_t = rpool.tile([P, nU], BF16)
                off = e["ch_off"]
                nc.gpsimd.tensor_tensor(out=r_t, in0=ch_all[:, off:off + nU],
                                        in1=v.to_broadcast((P, nU)), op=AL.mult)
                if e["c"] == "tgt":
                    pool, col_lo, stop = pool_t, e["ulist"][0] - 4, i == lt_tgt
                elif i <= lt_s1:
                    pool, col_lo, stop = pool_s, e["ulist"][0], i == lt_s1
                else:
                    assert e["ulist"] == [0]
                    pool, col_lo, stop = pool_s2, 0, i == lt_src
                nc.tensor.matmul(pool[:, col_lo:col_lo + nU], lhsT=cl_all[:, i, :],
                                 rhs=r_t, start=False, stop=stop, skip_group_check=True)
            if i == lt_tgt:
                # tgt tail early: broadcast row of the output, hidden under the
                # src-phase DMA stream
                mtgtm = segp.tile([P, 5], BF16)
                nc.vector.tensor_tensor(out=mtgtm, in0=pool_t, in1=rcT, op=AL.mult)
                rowb_ps = pp_row.tile([P, 512], F32)
                nc.tensor.matmul(rowb_ps[:, 0:127], lhsT=mtgtm[:, 0:1].to_broadcast((P, P)),
                                 rhs=ident[:, 1:128], start=True, stop=True)
                nc.tensor.matmul(rowb_ps[:, 127:255], lhsT=mtgtm[:, 1:2].to_broadcast((P, P)),
                                 rhs=ident, start=True, stop=True)
                nc.tensor.matmul(rowb_ps[:, 255:383], lhsT=mtgtm[:, 2:3].to_broadcast((P, P)),
                                 rhs=ident, start=True, stop=True)
                nc.tensor.matmul(rowb_ps[:, 383:511], lhsT=mtgtm[:, 3:4].to_broadcast((P, P)),
                                 rhs=ident, start=True, stop=True)
                nc.tensor.matmul(rowb_ps[:, 511:512], lhsT=mtgtm[:, 4:5].to_broadcast((P, P)),
                                 rhs=ident[:, 0:1], start=True, stop=True)
                nc.scalar.activation(out=rowb_sb, in_=rowb_ps, func=ACTF.Identity,
                                     bias=biascol, scale=1.0)
            if i == lt_s1:
                # pool_s closed: blocks 1-3 emitted under the x stream
                nc.vector.tensor_tensor(out=msrcm14, in0=pool_s, in1=rcS, op=AL.mult)
                for k in (1, 2, 3):
                    emit_block(k, msrcm14[:, k:k + 1], msrcm14[:, k + 1:k + 2])

        # ---- final tail: only block 0 (needs the trailing u=0 tiles) ----
        msrcm0 = segp.tile([P, 1], BF16)
        if lt_src > lt_s1:
            nc.vector.tensor_scalar(out=msrcm0, in0=pool_s[:, 0:1], scalar1=pool_s2,
                                    scalar2=rcS[:, 0:1], op0=AL.add, op1=AL.mult)
        else:
            nc.vector.tensor_tensor(out=msrcm0, in0=pool_s[:, 0:1], in1=rcS[:, 0:1],
                                    op=AL.mult)
        emit_block(0, msrcm0, msrcm14[:, 1:2])

    nc.compile()
    return nc


def _host_prep(inputs):
    x = np.asarray(inputs["outputs"], dtype=np.float32)
    wid = np.asarray(inputs["word_ids"]).astype(np.int64)
    cw = np.asarray(inputs["classifier_w"], dtype=np.float32)
    bias = np.float32(np.asarray(inputs["classifier_b"]))
    B, L, Hd = x.shape
    assert (Hd, L, B) == (H, 4096, 8)
    assert int(inputs["num_src"]) == 512 and int(inputs["num_tgt"]) == 512
    assert np.asarray(inputs["attention_mask"]).min() == 1

    segs, idxs = [], []
    for b in range(B):
        ns = np.ones(L, np.int64)
        ns[1:] = wid[b, 1:] != wid[b, :-1]
        seg = np.cumsum(ns) - 1
        keep = (seg >= 1) & (seg <= 1024)
        idxs.append(np.nonzero(keep)[0][::-1])  # descending segment order
        segs.append(seg)
    ntoks = [len(i) for i in idxs]
    NT = (max(ntoks) + P - 1) // P
    L2 = NT * P

    tok_s = np.full((B, L2), -1, np.int64)
    xbs = []
    for b in range(B):
        n = ntoks[b]
        tok_s[b, :n] = segs[b][idxs[b]]
        xi = np.zeros(L2, np.int64)
        xi[:n] = idxs[b]
        xbs.append(np.ascontiguousarray(x[b][xi]))

    is_t = tok_s >= 513
    is_s = (tok_s >= 1) & (tok_s <= 512)
    u = np.where(tok_s >= 0, tok_s >> 7, -1)
    slo_v = np.where(tok_s >= 0, tok_s & 127, -1)

    # program metadata, unioned over cores (same compiled program everywhere)
    ops, CW, ch_cols = [], 0, []
    for i in range(NT):
        sl = slice(i * P, (i + 1) * P)
        ent = []
        for cname, m in (("tgt", is_t), ("src", is_s)):
            msk = m[:, sl]
            if not msk.any():
                continue
            uu = u[:, sl][msk]
            ulist = list(range(int(uu.min()), int(uu.max()) + 1))
            assert len(ulist) <= 3
            d = dict(c=cname, ulist=ulist, ch_off=CW, red="act")
            for uv in ulist:
                ch_cols.append((i, cname, uv))
            CW += len(ulist)
            ent.append(d)
        ops.append(ent)
    lt_tgt = max(i for i in range(NT) if any(e["c"] == "tgt" for e in ops[i]))
    lt_src = max(i for i in range(NT) if any(e["c"] == "src" for e in ops[i]))
    lt_s1 = max(i for i in range(NT)
                if any(e["c"] == "src" and max(e["ulist"]) >= 1 for e in ops[i]))
    for i in range(lt_s1 + 1, NT):
        assert all(e["c"] == "src" and e["ulist"] == [0] for e in ops[i])
    # offload some reductions from ACT to DVE so neither engine exceeds the
    # DMA stream time
    flat = [e for ent in ops for e in ent]
    n = len(flat)
    # DVE takes ~6 reductions, spread through the middle so neither DVE nor
    # ACT carries a backlog past the end of the x stream
    for j in np.linspace(4, max(5, n - 6), 6).astype(int):
        if 0 <= j < n - 1:
            flat[int(j)]["red"] = "dve"

    iota_h = np.broadcast_to(np.arange(P, dtype=np.float32), (P, P))
    s1_h = np.eye(P, k=-1, dtype=np.float32)  # s1[p,m]=1 iff m==p-1 -> out[m]=in[m+1]
    s2_h = np.zeros((P, P), np.float32)
    s2_h[0, P - 1] = 1.0
    ident_h = np.eye(P, dtype=np.float32)
    wrep_h = np.broadcast_to(cw, (P, 2 * H)).astype(ml_dtypes.bfloat16)

    in_maps = []
    for b in range(B):
        cnt = np.bincount(tok_s[b][tok_s[b] >= 0], minlength=1025).astype(np.float64)
        rcS_h = np.ones((P, 5), np.float32)
        rcT_h = np.ones((P, 5), np.float32)
        for j in range(5):
            for p in range(P):
                s_src = 128 * j + p
                if 1 <= s_src <= 512:
                    rcS_h[p, j] = 1.0 / max(cnt[s_src], 1.0)
                s_tgt = 128 * (j + 4) + p
                if 513 <= s_tgt <= 1024:
                    rcT_h[p, j] = 1.0 / max(cnt[s_tgt], 1.0)
        slo_t = slo_v[b].reshape(NT, P).T.astype(np.float32)  # [128, NT]
        ch_h = np.zeros((P, CW), np.float32)
        for k, (i, cname, uv) in enumerate(ch_cols):
            m = (is_t if cname == "tgt" else is_s)[b, i * P:(i + 1) * P]
            ch_h[:, k] = (m & (u[b, i * P:(i + 1) * P] == uv)).astype(np.float32)
        biascol = np.full((P, 1), bias, np.float32)
        cf_h = np.concatenate([ch_h, rcS_h, rcT_h, biascol], axis=1)
        cb_h = np.concatenate(
            [s1_h, s2_h, ident_h, iota_h, slo_t, np.zeros((P, 8), np.float32)],
            axis=1).astype(ml_dtypes.bfloat16)
        in_maps.append({
            "x": xbs[b],
            "consts": np.ascontiguousarray(cf_h.astype(np.float32)),
            "cbf": np.ascontiguousarray(cb_h),
            "wrepb": np.ascontiguousarray(wrep_h),
        })
    return NT, ops, CW, lt_tgt, lt_s1, lt_src, in_maps


def _run(inputs, trace=False, tmpdir=None):
    NT, ops, CW, lt_tgt, lt_s1, lt_src, in_maps = _host_prep(inputs)
    nc = _build_nc(NT, ops, CW, lt_tgt, lt_s1, lt_src)
    res = run_bass_kernel_spmd(nc, in_maps, core_ids=list(range(8)), trace=trace, tmpdir=tmpdir)
    out = np.stack([np.asarray(r["y"]).astype(np.float32) for r in res.results])
    return out, res


def kernel(**inputs) -> np.ndarray:
    out, _ = _run(inputs, trace=False)
    return out


if __name__ == "__main__":
    # CoreSim smoke test on core 0's inputs
    import jax
    jax.config.update("jax_platforms", "cpu")
    sys.path.insert(0, "/root/problem")
    import reference as ref
    from concourse.bass_interp import CoreSim

    inputs = ref.setup_inputs()
    NT, ops, CW, lt_tgt, lt_s1, lt_src, in_maps = _host_prep(inputs)
    print("NT =", NT, "CW =", CW, "lt_tgt =", lt_tgt, "lt_s1 =", lt_s1, "lt_src =", lt_src)
    for i, ent in enumerate(ops):
        print(i, [(e["c"], e["ulist"], e["red"]) for e in ent])
    nc = _build_nc(NT, ops, CW, lt_tgt, lt_s1, lt_src)
    sim = CoreSim(nc)
    for name, arr in in_maps[0].items():
        sim.tensor(name)[:] = arr
    sim.simulate()
    got = np.array(sim.tensor("y")).astype(np.float32)
    expected = np.asarray(ref.reference(**inputs))[0]
    err = np.abs(got - expected).max()
    scale = np.abs(expected).max()
    print("CoreSim abs err:", err, "rel:", err / scale)
    assert err / scale < 1e-2, "CoreSim mismatch"
    print("CORESIM PASSES")
